# revision 48
# baseline (speedup 1.0000x reference)
"""Trainium2 Bass kernel for nn_CHSHistoryCrossAttentionFusion (8 NeuronCores, SPMD).

Decomposition (hardcoded for B=2, S=4096, L=3, D=1024, N=512, 8 cores):
  - History sequence-sharded: core c owns key positions [c*512, (c+1)*512) of
    each batch; it computes its chunk of fused/K/V from its x chunk.
  - Queries sharded 8-way for the Q path (64 batch-0 + 64 batch-1 queries per
    core); an AllGather replicates Q (bf16, small) so every core scores all
    1024 queries against its own K/V chunk.
  - Flash-style partial softmax per chunk WITHOUT max subtraction (Q/K are
    RMS-normalized so scores are bounded); causal mask applied additively
    before exp; exp carries a constant -ln(256) prescale so the (o,l)
    partials fit fp16.  Partials combine via four fp16 ReduceScatter-adds
    (two per batch, pipelined against attention compute); attn query tiles
    interleave all cores' queries 16-wise so each half-RS scatters rows the
    owning core actually wants.  Epilogue runs per 64-row batch half as its
    RS pair completes.
  - All matmuls bf16 (fp32 accumulate).  Activations/weights are shipped
    bf16 and pre-transposed/packed from host (pure layout+cast: the kernel
    consumed bf16 everywhere already), so the PE never transposes x and no
    f32 staging/casting happens on-chip.
  - PE-chain software pipelining: the transposes/attnV of tile t are issued
    after the matmul block of tile t+1 so the PE FIFO never waits on a
    pending rms/softmax chain.
Host-side work is layout/indexing/dtype-cast only.
"""

import math
import os

import numpy as np

try:
    import ml_dtypes
except ImportError:  # pragma: no cover
    ml_dtypes = None

import concourse.bacc as bacc
import concourse.mybir as mybir
import concourse.tile as tile
import concourse.tile_utils as tile_utils
from concourse.bass_utils import run_bass_kernel_spmd

tile_utils.max_sbuf_usage = 208 * 1024

F32 = mybir.dt.float32
F16 = mybir.dt.float16
BF16 = mybir.dt.bfloat16
AF = mybir.ActivationFunctionType
OP = mybir.AluOpType

B, S, L, D = 2, 4096, 3, 1024
N = 512
NC = 8
CH = S // NC              # 512 keys per batch per core
LD = L * D                # 3072
QT = B * N                # 1024 global queries
QPC = QT // NC            # 128 queries per core (64 per batch)
NKK = LD // 128           # 24 contraction slices over 3072
NJ = D // 128             # 8 contraction slices over 1024
RMS_EPS = 1e-6
SCALE = D ** -0.5
MASK_NEG = -60000.0          # fits f16; exp(SCALE*(s-6e4)) == 0 regardless
EXP_BIAS = -math.log(256.0)

_CACHE = {}


def _build(apply_norm_weights: bool):
    nc = bacc.Bacc("TRN2", target_bir_lowering=False, num_devices=NC)

    # ---------------- I/O (bf16 activations/weights) ----------------
    # xt: [128, b*12288 + kk*512 + tok]   (x chunk, transposed+packed)
    xt_d = nc.dram_tensor("xt", [128, B * NKK * CH], BF16, kind="ExternalInput")
    # xqt: [128, kk*128 + q]
    xqt_d = nc.dram_tensor("xqt", [128, NKK * 128], BF16, kind="ExternalInput")
    # wfc: [128, kk*1024 + d]
    wfc_d = nc.dram_tensor("wfc", [128, NKK * D], BF16, kind="ExternalInput")
    # wq/wk/wv/wo: [128, j*1024 + d]
    wq_d = nc.dram_tensor("wq", [128, NJ * D], BF16, kind="ExternalInput")
    wk_d = nc.dram_tensor("wk", [128, NJ * D], BF16, kind="ExternalInput")
    wv_d = nc.dram_tensor("wv", [128, NJ * D], BF16, kind="ExternalInput")
    wo_d = nc.dram_tensor("wo", [128, NJ * D], BF16, kind="ExternalInput")
    # pet: [128, j*512 + tok]  (positional table for this chunk, transposed)
    pet_d = nc.dram_tensor("pet", [128, NJ * CH], BF16, kind="ExternalInput")
    peq_d = nc.dram_tensor("peq", [QPC, D], BF16, kind="ExternalInput")
    thr_d = nc.dram_tensor("thr", [128, NC], F32, kind="ExternalInput")
    iota_d = nc.dram_tensor("iota", [128, CH], F16, kind="ExternalInput")
    ident_d = nc.dram_tensor("ident", [128, 128], BF16, kind="ExternalInput")
    if apply_norm_weights:
        whn_d = nc.dram_tensor("whn", [128, D], F32, kind="ExternalInput")
        wqn_d = nc.dram_tensor("wqn", [128, D], F32, kind="ExternalInput")
        wkn_d = nc.dram_tensor("wkn", [128, D], F32, kind="ExternalInput")
        won_d = nc.dram_tensor("won", [128, D], F32, kind="ExternalInput")
    out = nc.dram_tensor("out", [QPC, D], F32, kind="ExternalOutput")

    with tile.TileContext(nc) as tc:
        with (
            tc.tile_pool(name="dram", bufs=1, space="DRAM") as dram,
            tc.tile_pool(name="const", bufs=1) as constp,
            tc.tile_pool(name="stat", bufs=4) as stat,
            tc.tile_pool(name="base", bufs=1) as base,
            tc.tile_pool(name="scr_bf", bufs=2) as scr_bf,
            tc.tile_pool(name="scr_f", bufs=2) as scr_f,
            tc.tile_pool(name="mmps", bufs=2, space="PSUM") as mmps,
            tc.tile_pool(name="trps", bufs=2, space="PSUM") as trps,
            tc.tile_pool(name="scps", bufs=2, space="PSUM") as scps,
        ):
            # collective bounce buffers
            ag_in = dram.tile([QPC, D], BF16)
            ag_out = dram.tile([QT, D], BF16, addr_space="Shared")
            # batch 0 split 2+2 tiles (fires early, hidden); batch 1 split
            # 3+1 so only the small last RS sits on the critical tail
            RS_ROWS = [256, 256, 384, 128]
            rs_in = [dram.tile([RS_ROWS[h], D + 1], F16, name=f"rsin{h}")
                     for h in range(4)]
            rs_out = [dram.tile([RS_ROWS[h] // NC, D + 1], F16, name=f"rsout{h}")
                      for h in range(4)]
            # normalized fused (token-major) bounced via DRAM: written during
            # phase 1, read back as the attnV rhs in phase 2 (SBUF is full)
            fcache = dram.tile([B * 4 * 128, D], BF16)

            # constants (tiny, scalar queue)
            id_sb = constp.tile([128, 128], BF16)
            nc.scalar.dma_start(id_sb[:], ident_d.ap())
            id16_sb = constp.tile([128, 128], F16)
            nc.scalar.copy(id16_sb[:], id_sb[:])
            iota_sb = constp.tile([128, CH], F16)
            nc.scalar.dma_start(iota_sb[:], iota_d.ap())
            thr_sb = constp.tile([128, NC], F32)
            nc.scalar.dma_start(thr_sb[:], thr_d.ap())
            eps_sb = constp.tile([128, 1], F32)
            nc.vector.memset(eps_sb[:], RMS_EPS)
            ebias_sb = constp.tile([128, 1], F32)
            nc.vector.memset(ebias_sb[:], EXP_BIAS)
            if apply_norm_weights:
                whn_sb = constp.tile([128, D], F32)
                nc.scalar.dma_start(whn_sb[:], whn_d.ap())
                wqn_sb = constp.tile([128, D], F32)
                nc.scalar.dma_start(wqn_sb[:], wqn_d.ap())
                wkn_sb = constp.tile([128, D], F32)
                nc.scalar.dma_start(wkn_sb[:], wkn_d.ap())
                won_sb = constp.tile([128, D], F32)
                nc.scalar.dma_start(won_sb[:], won_d.ap())

            # persistent activations (DMAs issued inside ph1, ordered)
            fusedT_b = [base.tile([128, NJ * CH], BF16, name=f"fusedT{b}")
                        for b in range(B)]
            fusedT_bv = [fT[:].rearrange("p (j t) -> p j t", j=NJ)
                         for fT in fusedT_b]
            qs_f32 = base.tile([QPC, D], F32)
            wk_sb = base.tile([128, NJ * D], BF16)
            pet_sb = base.tile([128, NJ * CH], BF16)
            peq_sb = base.tile([QPC, D], BF16)

            def rms_stats(src_ap):
                sq = scr_f.tile([128, D], F32, tag="sqscr")
                ssq = stat.tile([128, 1], F32, tag="ssq")
                nc.scalar.activation(sq[:], src_ap, AF.Square, accum_out=ssq[:])
                std = stat.tile([128, 1], F32, tag="std")
                nc.scalar.activation(std[:], ssq[:], AF.Sqrt, scale=1.0 / D,
                                     bias=eps_sb[:])
                rstd = stat.tile([128, 1], F32, tag="rstd")
                nc.vector.reciprocal(rstd[:], std[:])
                return rstd

            def transpose_to(dst_ap_3d, src_tile_ap, jlist):
                """PE-transpose 128x128 blocks into dst 3d view [128,len,128]."""
                ps = trps.tile([128, 512], BF16, tag="trp")
                for u, j in enumerate(jlist):
                    nc.tensor.transpose(
                        ps[:, u * 128:(u + 1) * 128],
                        src_tile_ap[:, j * 128:(j + 1) * 128],
                        id_sb[:],
                    )
                nc.vector.tensor_copy(
                    dst_ap_3d,
                    ps[:].rearrange("p (u x) -> p u x", u=len(jlist)),
                )

            # =============== phase 1: fc matmuls (q tile first) ==============
            with tc.tile_pool(name="ph1", bufs=1) as ph1:
                wfc_sb = ph1.tile([128, NKK * D], BF16)
                for ck in range(4):
                    c0, c1 = ck * 6 * D, (ck + 1) * 6 * D
                    nc.scalar.dma_start(wfc_sb[:, c0:c1], wfc_d.ap()[:, c0:c1])
                xqt_sb = ph1.tile([128, NKK * 128], BF16)
                nc.sync.dma_start(xqt_sb[:], xqt_d.ap())
                wq_sb = ph1.tile([128, NJ * D], BF16)
                nc.sync.dma_start(wq_sb[:], wq_d.ap())
                nc.sync.dma_start(peq_sb[:], peq_d.ap())
                # xt packed tile-contiguous: col = (b*4+tl)*3072 + kk*128 + s
                # so each 128-token tile's FC can start as its chunk lands
                xt_sb = ph1.tile([128, B * NKK * CH], BF16)
                for ck in range(8):
                    c0, c1 = ck * NKK * 128, (ck + 1) * NKK * 128
                    nc.scalar.dma_start(xt_sb[:, c0:c1], xt_d.ap()[:, c0:c1])
                nc.scalar.dma_start(wk_sb[:], wk_d.ap())
                nc.scalar.dma_start(pet_sb[:], pet_d.ap())

                # pending PE work issued after the NEXT tile's matmul block
                # so the PE FIFO never waits on an rms chain
                pending = []

                def flush_pending():
                    while pending:
                        pending.pop(0)()

                def fc_tile(which):
                    fps = mmps.tile([128, D], F32, tag="mm")
                    if which[0] == 'q':
                        def lhs(kk):
                            return xqt_sb[:, kk * 128:(kk + 1) * 128]
                    else:
                        b, tl = which
                        c0 = (b * 4 + tl) * NKK * 128

                        def lhs(kk, c0=c0):
                            return xt_sb[:, c0 + kk * 128: c0 + kk * 128 + 128]
                    for kk in range(NKK):
                        for h in range(2):
                            nc.tensor.matmul(
                                fps[:, h * 512:(h + 1) * 512],
                                lhs(kk),
                                wfc_sb[:, kk * D + h * 512: kk * D + (h + 1) * 512],
                                start=(kk == 0),
                                stop=(kk == NKK - 1),
                            )
                    flush_pending()
                    rstd = rms_stats(fps[:])
                    fb = scr_bf.tile([128, D], BF16, tag="tmb")
                    nc.vector.tensor_scalar(fb[:], fps[:], rstd[:], None, OP.mult)
                    if apply_norm_weights:
                        nc.vector.tensor_tensor(fb[:], fb[:], whn_sb[:],
                                                op=OP.mult)
                    if which[0] == 'q':
                        nc.vector.tensor_scalar(qs_f32[:], fps[:], rstd[:],
                                                None, OP.mult)
                        if apply_norm_weights:
                            nc.vector.tensor_tensor(qs_f32[:], qs_f32[:],
                                                    whn_sb[:], op=OP.mult)
                        qhb = scr_bf.tile([128, D], BF16, tag="tmb")
                        nc.vector.tensor_add(qhb[:], fb[:], peq_sb[:])

                        def do_q_proj(qhb=qhb):
                            qht = scr_bf.tile([128, D], BF16, tag="tmb")
                            qht_v = qht[:].rearrange("p (g x) -> p g x", g=2)
                            for g in range(2):
                                transpose_to(
                                    qht_v[:, g:g + 1, :]
                                    .rearrange("p g x -> p (g x)")
                                    .rearrange("p (u x) -> p u x", u=4),
                                    qhb[:],
                                    [g * 4 + u for u in range(4)],
                                )
                            qps = mmps.tile([128, D], F32, tag="mm")
                            for j in range(NJ):
                                for h in range(2):
                                    nc.tensor.matmul(
                                        qps[:, h * 512:(h + 1) * 512],
                                        qht[:, j * 128:(j + 1) * 128],
                                        wq_sb[:, j * D + h * 512:
                                              j * D + (h + 1) * 512],
                                        start=(j == 0),
                                        stop=(j == NJ - 1),
                                    )
                            qrstd = rms_stats(qps[:])
                            qb = scr_bf.tile([128, D], BF16, tag="tmb")
                            nc.vector.tensor_scalar(qb[:], qps[:], qrstd[:],
                                                    None, OP.mult)
                            if apply_norm_weights:
                                nc.vector.tensor_tensor(qb[:], qb[:], wqn_sb[:],
                                                        op=OP.mult)
                            nc.sync.dma_start(ag_in[:], qb[:])
                            nc.gpsimd.collective_compute(
                                "AllGather", OP.bypass,
                                replica_groups=[list(range(NC))],
                                ins=[ag_in.opt()],
                                outs=[ag_out.opt()],
                            )
                        pending.append(do_q_proj)
                    else:
                        b, tl = which
                        # gpsimd DMA queue: keeps the scalar FIFO free so the
                        # next tile's rms Square isn't gated by this tile's fb
                        nc.gpsimd.dma_start(
                            fcache[(b * 4 + tl) * 128:(b * 4 + tl + 1) * 128, :],
                            fb[:])

                        def do_f_trans(b=b, tl=tl, fb=fb):
                            for g in range(2):
                                transpose_to(
                                    fusedT_bv[b][:, g * 4:(g + 1) * 4,
                                                 tl * 128:(tl + 1) * 128],
                                    fb[:],
                                    [g * 4 + u for u in range(4)],
                                )
                        pending.append(do_f_trans)

                fc_tile(('q',))
                flush_pending()   # q projection + AllGather fire immediately
                for b in range(B):
                    for tl in range(4):
                        fc_tile((b, tl))
                flush_pending()

            # =============== phase 2: K/V, attention, RS, epilogue ===========
            with tc.tile_pool(name="ph2", bufs=1) as ph2:
                kT_b = [ph2.tile([128, NJ * CH], BF16, name=f"kT{b}")
                        for b in range(B)]
                kT_bv = [kT[:].rearrange("p (j t) -> p j t", j=NJ)
                         for kT in kT_b]
                # token-major normalized fused, read back from DRAM: the
                # attnV rhs (V projection is folded into the epilogue since
                # (P @ fused) @ Wv == P @ (fused @ Wv))
                ftok_b = [ph2.tile([128, 4 * D], BF16, name=f"ftok{b}")
                          for b in range(B)]
                for b in range(B):
                    for tl in range(4):
                        nc.scalar.dma_start(
                            ftok_b[b][:, tl * D:(tl + 1) * D],
                            fcache[(b * 4 + tl) * 128:(b * 4 + tl + 1) * 128, :])
                qT = ph2.tile([128, NJ * QT], BF16)
                qT_v = qT[:].rearrange("p (j t) -> p j t", j=NJ)
                wv_sb = ph2.tile([128, NJ * D], BF16)
                nc.scalar.dma_start(wv_sb[:], wv_d.ap())
                wo_sb = ph2.tile([128, NJ * D], BF16)
                nc.scalar.dma_start(wo_sb[:], wo_d.ap())

                # precompute all 8 causal masks (depends only on iota/thr);
                # each is added onto the scores INSIDE the PSUM accumulation
                # via an identity matmul, so softmax needs no vector hop
                mb_all = ph2.tile([128, NC * CH], BF16)
                for ti in range(NC):
                    nc.vector.tensor_scalar(mb_all[:, ti * CH:(ti + 1) * CH],
                                            iota_sb[:], thr_sb[:, ti:ti + 1],
                                            MASK_NEG, OP.is_gt, OP.mult)

                pend2 = []

                def flush2():
                    while pend2:
                        pend2.pop(0)()

                def flush_keep(n):
                    while len(pend2) > n:
                        pend2.pop(0)()

                def k_tile(bb, tl):
                    khb = scr_bf.tile([128, NJ * 128], BF16, tag="khb")
                    nc.vector.tensor_add(
                        khb[:].rearrange("p (j x) -> p j x", j=NJ),
                        fusedT_bv[bb][:, :, tl * 128:(tl + 1) * 128],
                        pet_sb[:].rearrange("p (j t) -> p j t", j=NJ)
                        [:, :, tl * 128:(tl + 1) * 128],
                    )
                    kps = mmps.tile([128, D], F32, tag="mm")
                    for j in range(NJ):
                        for h in range(2):
                            nc.tensor.matmul(
                                kps[:, h * 512:(h + 1) * 512],
                                khb[:, j * 128:(j + 1) * 128],
                                wk_sb[:, j * D + h * 512: j * D + (h + 1) * 512],
                                start=(j == 0),
                                stop=(j == NJ - 1),
                            )
                    flush2()
                    krstd = rms_stats(kps[:])
                    kb = scr_bf.tile([128, D], BF16, tag="tmb")
                    nc.vector.tensor_scalar(kb[:], kps[:], krstd[:], None, OP.mult)
                    if apply_norm_weights:
                        nc.vector.tensor_tensor(kb[:], kb[:], wkn_sb[:],
                                                op=OP.mult)

                    def do_k_trans(bb=bb, tl=tl, kb=kb):
                        for g in range(2):
                            transpose_to(
                                kT_bv[bb][:, g * 4:(g + 1) * 4,
                                          tl * 128:(tl + 1) * 128],
                                kb[:],
                                [g * 4 + u for u in range(4)],
                            )
                    pend2.append(do_k_trans)

                # ag_out rows viewed as (core, 16-query block, slot)
                ag_v = ag_out[:].rearrange("(c k s) d -> c k s d", c=NC, k=8)

                def qt_tile(ti):
                    """Gather attn q-tile ti (16 queries from each core) in
                    one strided DMA (sync-engine descriptor time is scarce)."""
                    b, i = divmod(ti, 4)
                    qg = scr_bf.tile([128, D], BF16, tag="qg", bufs=2)
                    nc.sync.dma_start(qg[:], ag_v[:, b * 4 + i, :, :])

                    def do_qt_trans(ti=ti, qg=qg):
                        for g in range(2):
                            transpose_to(
                                qT_v[:, g * 4:(g + 1) * 4,
                                     ti * 128:(ti + 1) * 128],
                                qg[:],
                                [g * 4 + u for u in range(4)],
                            )
                    pend2.append(do_qt_trans)

                # attn tile -> (rs buffer, per-core row stride, row offset)
                RS_MAP = {(0, 0): (0, 32, 0), (0, 1): (0, 32, 16),
                          (0, 2): (1, 32, 0), (0, 3): (1, 32, 16),
                          (1, 0): (2, 48, 0), (1, 1): (2, 48, 16),
                          (1, 2): (2, 48, 32), (1, 3): (3, 16, 0)}

                def attn_tile(bb, i):
                    ti = bb * 4 + i
                    sps = scps.tile([128, 512], F32, tag="sc")
                    for j in range(NJ):
                        nc.tensor.matmul(
                            sps[:],
                            qT[:, j * QT + ti * 128: j * QT + (ti + 1) * 128],
                            kT_b[bb][:, j * CH:(j + 1) * CH],
                            start=(j == 0),
                            stop=False,
                        )
                    nc.tensor.matmul(          # sps += I @ mask
                        sps[:],
                        id_sb[:],
                        mb_all[:, ti * CH:(ti + 1) * CH],
                        start=False,
                        stop=True,
                    )
                    flush_keep(2)
                    o_sb = scr_f.tile([128, D + 1], F16, tag="osb")
                    lacc = stat.tile([128, 1], F32, tag="lacc")
                    probs = scr_bf.tile([128, CH], BF16, tag="probs")
                    nc.scalar.activation(probs[:], sps[:], AF.Exp, scale=SCALE,
                                         bias=ebias_sb[:], accum_out=lacc[:])
                    nc.vector.tensor_copy(o_sb[:, D:D + 1], lacc[:])

                    def do_attn_out(bb=bb, i=i, probs=probs, o_sb=o_sb):
                        pps = trps.tile([128, 512], BF16, tag="trp")
                        for u in range(4):
                            nc.tensor.transpose(
                                pps[:, u * 128:(u + 1) * 128],
                                probs[:, u * 128:(u + 1) * 128],
                                id_sb[:],
                            )
                        pT = scr_bf.tile([128, 512], BF16, tag="pT")
                        nc.vector.tensor_copy(pT[:], pps[:])
                        ops_ = mmps.tile([128, D], F32, tag="mm")
                        for u in range(4):
                            for h in range(2):
                                nc.tensor.matmul(
                                    ops_[:, h * 512:(h + 1) * 512],
                                    pT[:, u * 128:(u + 1) * 128],
                                    ftok_b[bb][:, u * D + h * 512:
                                               u * D + h * 512 + 512],
                                    start=(u == 0),
                                    stop=(u == 3),
                                )
                        nc.vector.tensor_copy(o_sb[:, 0:D], ops_[:])
                        ri, stride, off = RS_MAP[(bb, i)]
                        dst = rs_in[ri][:].rearrange(
                            "(c o s) d -> c o s d", c=NC, o=stride // 16)
                        nc.sync.dma_start(
                            dst[:, off // 16, :, :],
                            o_sb[:].rearrange("(c s) d -> c s d", c=NC))
                    pend2.append(do_attn_out)

                def reduce_scatter(h):
                    nc.gpsimd.collective_compute(
                        "ReduceScatter", OP.add,
                        replica_groups=[list(range(NC))],
                        ins=[rs_in[h].opt()],
                        outs=[rs_out[h].opt()],
                    )

                def epilogue_half(bb):
                    # everything lives on partitions [bb*64, bb*64+64) so all
                    # elementwise ops have matching start partitions
                    p0 = bb * 64
                    sl = slice(p0, p0 + 64)
                    fo = scr_f.tile([128, D + 1], F16, tag="fo")
                    r0 = RS_ROWS[bb * 2] // NC
                    r1 = RS_ROWS[bb * 2 + 1] // NC
                    nc.sync.dma_start(fo[p0:p0 + r0, :], rs_out[bb * 2])
                    nc.sync.dma_start(fo[p0 + r0:p0 + r0 + r1, :],
                                      rs_out[bb * 2 + 1])
                    linv = stat.tile([128, 1], F32, tag="linv")
                    nc.vector.reciprocal(linv[sl, :], fo[sl, D:D + 1])

                    def trans64(dst, src, idm=id_sb, ps_dt=BF16):
                        for g in range(2):
                            ps = trps.tile([128, 512], ps_dt, tag="trp")
                            for u in range(4):
                                j = g * 4 + u
                                nc.tensor.transpose(
                                    ps[:, u * 64:(u + 1) * 64],
                                    src[sl, j * 128:(j + 1) * 128],
                                    idm[sl, p0:p0 + 64],
                                )
                            nc.vector.tensor_copy(dst[:, g * 256:(g + 1) * 256],
                                                  ps[:, 0:256])

                    aoT = scr_bf.tile([128, 512], BF16, tag="pT")
                    trans64(aoT, fo, idm=id16_sb, ps_dt=F16)
                    # V projection folded out of the per-chunk attention:
                    # apply Wv then Wo to this core's 64 rows only
                    z1 = mmps.tile([128, D], F32, tag="mm")
                    for j in range(NJ):
                        for h in range(2):
                            nc.tensor.matmul(
                                z1[sl, h * 512:(h + 1) * 512],
                                aoT[:, j * 64:(j + 1) * 64],
                                wv_sb[:, j * D + h * 512: j * D + (h + 1) * 512],
                                start=(j == 0),
                                stop=(j == NJ - 1),
                            )
                    z1b = scr_bf.tile([128, D], BF16, tag="tmb")
                    nc.scalar.copy(z1b[sl, :], z1[sl, :])
                    z1T = scr_bf.tile([128, 512], BF16, tag="pT")
                    trans64(z1T, z1b)
                    zps = mmps.tile([128, D], F32, tag="mm")
                    for j in range(NJ):
                        for h in range(2):
                            nc.tensor.matmul(
                                zps[sl, h * 512:(h + 1) * 512],
                                z1T[:, j * 64:(j + 1) * 64],
                                wo_sb[:, j * D + h * 512: j * D + (h + 1) * 512],
                                start=(j == 0),
                                stop=(j == NJ - 1),
                            )
                    # divide by l after Wo (row scaling commutes with @Wo)
                    hh = scr_f.tile([128, D], F32, tag="sqscr")
                    nc.vector.tensor_scalar(hh[sl, :], zps[sl, :], linv[sl, :],
                                            None, OP.mult)
                    nc.vector.tensor_tensor(hh[sl, :], hh[sl, :],
                                            qs_f32[sl, :], op=OP.add)
                    sq = scr_f.tile([128, D], F32, tag="sqscr")
                    ssq = stat.tile([128, 1], F32, tag="ssq2")
                    nc.scalar.activation(sq[sl, :], hh[sl, :], AF.Square,
                                         accum_out=ssq[sl, :])
                    ostd = stat.tile([128, 1], F32, tag="std2")
                    nc.scalar.activation(ostd[sl, :], ssq[sl, :], AF.Sqrt,
                                         scale=1.0 / D, bias=eps_sb[sl, :])
                    orstd = stat.tile([128, 1], F32, tag="rstd2")
                    nc.vector.reciprocal(orstd[sl, :], ostd[sl, :])
                    yv = scr_f.tile([128, D], F32, tag="yv", bufs=1)
                    nc.vector.tensor_scalar(yv[sl, :], hh[sl, :], orstd[sl, :],
                                            None, OP.mult)
                    if apply_norm_weights:
                        nc.vector.tensor_tensor(yv[sl, :], yv[sl, :],
                                                won_sb[sl, :], op=OP.mult)
                    nc.sync.dma_start(out.ap()[p0:p0 + 64, :], yv[sl, :])

                # ---- schedule ----
                # all K tiles first (scores only need kT), then V, so the
                # attention block (and its RS triggers) starts ~15us earlier
                for tl in range(4):
                    k_tile(0, tl)
                    qt_tile(2 * tl)
                    qt_tile(2 * tl + 1)
                flush2()        # kT(0,3) + qT tiles 6,7 before attn scores
                attn_tile(0, 0)
                attn_tile(0, 1)
                flush2()
                reduce_scatter(0)
                attn_tile(0, 2)
                attn_tile(0, 3)
                flush2()
                reduce_scatter(1)
                for tl in range(4):
                    k_tile(1, tl)
                flush2()        # kT(1,3) before batch-1 attn scores
                attn_tile(1, 0)
                attn_tile(1, 1)
                attn_tile(1, 2)
                flush2()
                reduce_scatter(2)
                attn_tile(1, 3)
                flush2()
                reduce_scatter(3)
                epilogue_half(0)   # PE work overlaps RS(3) wait
                epilogue_half(1)

    nc.compile()
    return nc


def _pe_table():
    half = D // 2
    inv_freq = np.exp(np.arange(half, dtype=np.float32)
                      * (-math.log(10000.0) / half))
    ang = np.arange(S, dtype=np.float32)[:, None] * inv_freq
    return np.concatenate([np.sin(ang), np.cos(ang)], axis=-1).astype(np.float32)


def _core_gidx(c):
    """Global query indices owned by core c (64 batch-0 then 64 batch-1)."""
    h = QPC // 2
    return np.concatenate([np.arange(c * h, (c + 1) * h),
                           N + np.arange(c * h, (c + 1) * h)])


def _pack_rows(a, nrow=128):
    """[R, C] with R = k*nrow -> [nrow, k*C] (block k at cols k*C)."""
    k = a.shape[0] // nrow
    return np.ascontiguousarray(
        a.reshape(k, nrow, a.shape[1]).transpose(1, 0, 2).reshape(nrow, -1))


def make_in_maps(np_inputs, apply_w=False):
    bf16 = ml_dtypes.bfloat16
    hid = np.asarray(np_inputs["hidden_states"], np.float32)
    pos = np.asarray(np_inputs["context_positions"])
    Wfc = np.asarray(np_inputs["W_fc"], np.float32)
    Wq = np.asarray(np_inputs["Wq"], np.float32)
    Wk = np.asarray(np_inputs["Wk"], np.float32)
    Wv = np.asarray(np_inputs["Wv"], np.float32)
    Wo = np.asarray(np_inputs["Wo"], np.float32)

    x = hid.reshape(B, S, LD)
    p = np.clip(pos.astype(np.int64), 0, S - 1)
    p_flat = p.reshape(QT)
    PE = _pe_table()

    wfc_p = _pack_rows(Wfc).astype(bf16)          # [128, 24*1024]
    wq_p = _pack_rows(Wq).astype(bf16)
    wk_p = _pack_rows(Wk).astype(bf16)
    wv_p = _pack_rows(Wv).astype(bf16)
    wo_p = _pack_rows(Wo).astype(bf16)

    iota_np = np.tile(np.arange(CH, dtype=np.float16), (128, 1))
    ident_np = np.eye(128, dtype=np.float32).astype(bf16)

    # thr[r, b*4+i] = position of query slot r in attn tile (b, i), minus
    # the core's chunk start.  Slot r = core (r//16)'s query (i*16 + r%16).
    rr = np.arange(128)

    in_maps = []
    for c in range(NC):
        sl = slice(c * CH, (c + 1) * CH)
        # xt col = (b*4+tl)*3072 + kk*128 + s  (tile-contiguous)
        xt_a = np.concatenate(
            [x[b, sl].reshape(4, 128, NKK, 128).transpose(3, 0, 2, 1)
             .reshape(128, NKK * CH) for b in range(B)],
            axis=1).astype(bf16)
        gidx = _core_gidx(c)
        xq_rows = x[gidx // N, p_flat[gidx]]              # [128, 3072]
        xqt_a = _pack_rows(np.ascontiguousarray(xq_rows.T)).astype(bf16)
        peq_a = PE[p_flat[gidx]].astype(bf16)
        pet_a = _pack_rows(np.ascontiguousarray(PE[sl].T)).astype(bf16)
        thr_a = np.empty((128, NC), np.float32)
        for b in range(B):
            for i in range(4):
                qidx = b * N + (rr // 16) * 64 + i * 16 + (rr % 16)
                thr_a[:, b * 4 + i] = p_flat[qidx].astype(np.float32) - c * CH
        m = {
            "xt": xt_a, "xqt": xqt_a,
            "wfc": wfc_p, "wq": wq_p, "wk": wk_p, "wv": wv_p, "wo": wo_p,
            "pet": pet_a, "peq": peq_a, "thr": thr_a,
            "iota": iota_np, "ident": ident_np,
        }
        if apply_w:
            m["whn"] = np.tile(np.asarray(np_inputs["w_hidden_norm"], np.float32), (128, 1))
            m["wqn"] = np.tile(np.asarray(np_inputs["w_q_norm"], np.float32), (128, 1))
            m["wkn"] = np.tile(np.asarray(np_inputs["w_k_norm"], np.float32), (128, 1))
            m["won"] = np.tile(np.asarray(np_inputs["w_out_norm"], np.float32), (128, 1))
        in_maps.append(m)
    return in_maps


def assemble_out(results):
    y = np.zeros((QT, D), np.float32)
    for c in range(NC):
        y[_core_gidx(c)] = results[c]["out"]
    return y.reshape(B, N, D)


def kernel(**inputs) -> np.ndarray:
    w_h = np.asarray(inputs["w_hidden_norm"], np.float32)
    w_q = np.asarray(inputs["w_q_norm"], np.float32)
    w_k = np.asarray(inputs["w_k_norm"], np.float32)
    w_o = np.asarray(inputs["w_out_norm"], np.float32)
    apply_w = not (np.all(w_h == 1) and np.all(w_q == 1)
                   and np.all(w_k == 1) and np.all(w_o == 1))

    key = ("nc", apply_w)
    if key not in _CACHE:
        _CACHE[key] = _build(apply_w)
    nc = _CACHE[key]

    in_maps = make_in_maps(inputs, apply_w)

    trace = os.environ.get("KERNEL_TRACE", "0") == "1"
    if trace:
        try:
            import axon_prof
            axon_prof.install()
        except Exception:
            trace = False
    res = run_bass_kernel_spmd(nc, in_maps, list(range(NC)), trace=trace)
    global LAST_EXEC_NS
    LAST_EXEC_NS = res.exec_time_ns

    return assemble_out(res.results).astype(np.float32)


LAST_EXEC_NS = None


# revision 49
# speedup vs baseline: 1.0381x; 1.0381x over previous
"""Trainium2 Bass kernel for nn_CHSHistoryCrossAttentionFusion (8 NeuronCores, SPMD).

Decomposition (hardcoded for B=2, S=4096, L=3, D=1024, N=512, 8 cores):
  - History sequence-sharded: core c owns key positions [c*512, (c+1)*512) of
    each batch; it computes its chunk of fused/K/V from its x chunk.
  - Queries sharded 8-way for the Q path (64 batch-0 + 64 batch-1 queries per
    core); an AllGather replicates Q (bf16, small) so every core scores all
    1024 queries against its own K/V chunk.
  - Flash-style partial softmax per chunk WITHOUT max subtraction (Q/K are
    RMS-normalized so scores are bounded); causal mask applied additively
    before exp; exp carries a constant -ln(256) prescale so the (o,l)
    partials fit fp16.  Partials combine via four fp16 ReduceScatter-adds
    (two per batch, pipelined against attention compute); attn query tiles
    interleave all cores' queries 16-wise so each half-RS scatters rows the
    owning core actually wants.  Epilogue runs per 64-row batch half as its
    RS pair completes.
  - All matmuls bf16 (fp32 accumulate).  Activations/weights are shipped
    bf16 and pre-transposed/packed from host (pure layout+cast: the kernel
    consumed bf16 everywhere already), so the PE never transposes x and no
    f32 staging/casting happens on-chip.
  - PE-chain software pipelining: the transposes/attnV of tile t are issued
    after the matmul block of tile t+1 so the PE FIFO never waits on a
    pending rms/softmax chain.
Host-side work is layout/indexing/dtype-cast only.
"""

import math
import os

import numpy as np

try:
    import ml_dtypes
except ImportError:  # pragma: no cover
    ml_dtypes = None

import concourse.bacc as bacc
import concourse.mybir as mybir
import concourse.tile as tile
import concourse.tile_utils as tile_utils
from concourse.bass_utils import run_bass_kernel_spmd

tile_utils.max_sbuf_usage = 208 * 1024

F32 = mybir.dt.float32
F16 = mybir.dt.float16
BF16 = mybir.dt.bfloat16
AF = mybir.ActivationFunctionType
OP = mybir.AluOpType

B, S, L, D = 2, 4096, 3, 1024
N = 512
NC = 8
CH = S // NC              # 512 keys per batch per core
LD = L * D                # 3072
QT = B * N                # 1024 global queries
QPC = QT // NC            # 128 queries per core (64 per batch)
NKK = LD // 128           # 24 contraction slices over 3072
NJ = D // 128             # 8 contraction slices over 1024
RMS_EPS = 1e-6
SCALE = D ** -0.5
MASK_NEG = -60000.0          # fits f16; exp(SCALE*(s-6e4)) == 0 regardless
EXP_BIAS = -math.log(256.0)

_CACHE = {}


def _build(apply_norm_weights: bool):
    nc = bacc.Bacc("TRN2", target_bir_lowering=False, num_devices=NC)

    # ---------------- I/O (bf16 activations/weights) ----------------
    # xt: [128, b*12288 + kk*512 + tok]   (x chunk, transposed+packed)
    xt_d = nc.dram_tensor("xt", [128, B * NKK * CH], BF16, kind="ExternalInput")
    # xqt: [128, kk*128 + q]
    xqt_d = nc.dram_tensor("xqt", [128, NKK * 128], BF16, kind="ExternalInput")
    # wfc: [128, kk*1024 + d]
    wfc_d = nc.dram_tensor("wfc", [128, NKK * D], BF16, kind="ExternalInput")
    # wq/wk/wv/wo: [128, j*1024 + d]
    wq_d = nc.dram_tensor("wq", [128, NJ * D], BF16, kind="ExternalInput")
    wk_d = nc.dram_tensor("wk", [128, NJ * D], BF16, kind="ExternalInput")
    wv_d = nc.dram_tensor("wv", [128, NJ * D], BF16, kind="ExternalInput")
    wo_d = nc.dram_tensor("wo", [128, NJ * D], BF16, kind="ExternalInput")
    # pet: [128, j*512 + tok]  (positional table for this chunk, transposed)
    pet_d = nc.dram_tensor("pet", [128, NJ * CH], BF16, kind="ExternalInput")
    peq_d = nc.dram_tensor("peq", [QPC, D], BF16, kind="ExternalInput")
    thr_d = nc.dram_tensor("thr", [128, NC], F32, kind="ExternalInput")
    iota_d = nc.dram_tensor("iota", [128, CH], F16, kind="ExternalInput")
    ident_d = nc.dram_tensor("ident", [128, 128], BF16, kind="ExternalInput")
    if apply_norm_weights:
        whn_d = nc.dram_tensor("whn", [128, D], F32, kind="ExternalInput")
        wqn_d = nc.dram_tensor("wqn", [128, D], F32, kind="ExternalInput")
        wkn_d = nc.dram_tensor("wkn", [128, D], F32, kind="ExternalInput")
        won_d = nc.dram_tensor("won", [128, D], F32, kind="ExternalInput")
    out = nc.dram_tensor("out", [QPC, D], F32, kind="ExternalOutput")

    with tile.TileContext(nc) as tc:
        with (
            tc.tile_pool(name="dram", bufs=1, space="DRAM") as dram,
            tc.tile_pool(name="const", bufs=1) as constp,
            tc.tile_pool(name="stat", bufs=4) as stat,
            tc.tile_pool(name="base", bufs=1) as base,
            tc.tile_pool(name="scr_bf", bufs=2) as scr_bf,
            tc.tile_pool(name="scr_f", bufs=2) as scr_f,
            tc.tile_pool(name="mmps", bufs=2, space="PSUM") as mmps,
            tc.tile_pool(name="trps", bufs=2, space="PSUM") as trps,
            tc.tile_pool(name="scps", bufs=2, space="PSUM") as scps,
        ):
            # collective bounce buffers
            ag_in = dram.tile([QPC, D], BF16)
            ag_out = dram.tile([QT, D], BF16, addr_space="Shared")
            # batch 0 split 2+2 tiles (fires early, hidden); batch 1 split
            # 3+1 so only the small last RS sits on the critical tail
            RS_ROWS = [256, 256, 384, 128]
            rs_in = [dram.tile([RS_ROWS[h], D + 1], F16, name=f"rsin{h}")
                     for h in range(4)]
            rs_out = [dram.tile([RS_ROWS[h] // NC, D + 1], F16, name=f"rsout{h}")
                      for h in range(4)]
            # normalized fused (token-major) bounced via DRAM: written during
            # phase 1, read back as the attnV rhs in phase 2 (SBUF is full)
            fcache = dram.tile([B * 4 * 128, D], BF16)

            # constants (tiny, scalar queue)
            id_sb = constp.tile([128, 128], BF16)
            nc.scalar.dma_start(id_sb[:], ident_d.ap())
            id16_sb = constp.tile([128, 128], F16)
            nc.scalar.copy(id16_sb[:], id_sb[:])
            iota_sb = constp.tile([128, CH], F16)
            nc.scalar.dma_start(iota_sb[:], iota_d.ap())
            thr_sb = constp.tile([128, NC], F32)
            nc.scalar.dma_start(thr_sb[:], thr_d.ap())
            eps_sb = constp.tile([128, 1], F32)
            nc.vector.memset(eps_sb[:], RMS_EPS)
            ebias_sb = constp.tile([128, 1], F32)
            nc.vector.memset(ebias_sb[:], EXP_BIAS)
            if apply_norm_weights:
                whn_sb = constp.tile([128, D], F32)
                nc.scalar.dma_start(whn_sb[:], whn_d.ap())
                wqn_sb = constp.tile([128, D], F32)
                nc.scalar.dma_start(wqn_sb[:], wqn_d.ap())
                wkn_sb = constp.tile([128, D], F32)
                nc.scalar.dma_start(wkn_sb[:], wkn_d.ap())
                won_sb = constp.tile([128, D], F32)
                nc.scalar.dma_start(won_sb[:], won_d.ap())

            # persistent activations (DMAs issued inside ph1, ordered)
            fusedT_b = [base.tile([128, NJ * CH], BF16, name=f"fusedT{b}")
                        for b in range(B)]
            fusedT_bv = [fT[:].rearrange("p (j t) -> p j t", j=NJ)
                         for fT in fusedT_b]
            qs_f32 = base.tile([QPC, D], F32)
            wk_sb = base.tile([128, NJ * D], BF16)
            pet_sb = base.tile([128, NJ * CH], BF16)
            peq_sb = base.tile([QPC, D], BF16)

            def rms_stats(src_ap):
                sq = scr_f.tile([128, D], F32, tag="sqscr")
                ssq = stat.tile([128, 1], F32, tag="ssq")
                nc.scalar.activation(sq[:], src_ap, AF.Square, accum_out=ssq[:])
                std = stat.tile([128, 1], F32, tag="std")
                nc.scalar.activation(std[:], ssq[:], AF.Sqrt, scale=1.0 / D,
                                     bias=eps_sb[:])
                rstd = stat.tile([128, 1], F32, tag="rstd")
                nc.vector.reciprocal(rstd[:], std[:])
                return rstd

            def transpose_to(dst_ap_3d, src_tile_ap, jlist):
                """PE-transpose 128x128 blocks into dst 3d view [128,len,128]."""
                ps = trps.tile([128, 512], BF16, tag="trp")
                for u, j in enumerate(jlist):
                    nc.tensor.transpose(
                        ps[:, u * 128:(u + 1) * 128],
                        src_tile_ap[:, j * 128:(j + 1) * 128],
                        id_sb[:],
                    )
                nc.vector.tensor_copy(
                    dst_ap_3d,
                    ps[:].rearrange("p (u x) -> p u x", u=len(jlist)),
                )

            # =============== phase 1: fc matmuls (q tile first) ==============
            with tc.tile_pool(name="ph1", bufs=1) as ph1:
                wfc_sb = ph1.tile([128, NKK * D], BF16)
                for ck in range(4):
                    c0, c1 = ck * 6 * D, (ck + 1) * 6 * D
                    nc.scalar.dma_start(wfc_sb[:, c0:c1], wfc_d.ap()[:, c0:c1])
                xqt_sb = ph1.tile([128, NKK * 128], BF16)
                nc.sync.dma_start(xqt_sb[:], xqt_d.ap())
                wq_sb = ph1.tile([128, NJ * D], BF16)
                nc.sync.dma_start(wq_sb[:], wq_d.ap())
                nc.sync.dma_start(peq_sb[:], peq_d.ap())
                # xt packed tile-contiguous: col = (b*4+tl)*3072 + kk*128 + s
                # so each 128-token tile's FC can start as its chunk lands
                xt_sb = ph1.tile([128, B * NKK * CH], BF16)
                for ck in range(8):
                    c0, c1 = ck * NKK * 128, (ck + 1) * NKK * 128
                    nc.scalar.dma_start(xt_sb[:, c0:c1], xt_d.ap()[:, c0:c1])
                nc.scalar.dma_start(wk_sb[:], wk_d.ap())
                nc.scalar.dma_start(pet_sb[:], pet_d.ap())

                # pending PE work issued after the NEXT tile's matmul block
                # so the PE FIFO never waits on an rms chain
                pending = []

                def flush_pending():
                    while pending:
                        pending.pop(0)()

                def fc_tile(which):
                    fps = mmps.tile([128, D], F32, tag="mm")
                    if which[0] == 'q':
                        def lhs(kk):
                            return xqt_sb[:, kk * 128:(kk + 1) * 128]
                    else:
                        b, tl = which
                        c0 = (b * 4 + tl) * NKK * 128

                        def lhs(kk, c0=c0):
                            return xt_sb[:, c0 + kk * 128: c0 + kk * 128 + 128]
                    for kk in range(NKK):
                        for h in range(2):
                            nc.tensor.matmul(
                                fps[:, h * 512:(h + 1) * 512],
                                lhs(kk),
                                wfc_sb[:, kk * D + h * 512: kk * D + (h + 1) * 512],
                                start=(kk == 0),
                                stop=(kk == NKK - 1),
                            )
                    flush_pending()
                    rstd = rms_stats(fps[:])
                    fb = scr_bf.tile([128, D], BF16, tag="tmb")
                    nc.vector.tensor_scalar(fb[:], fps[:], rstd[:], None, OP.mult)
                    if apply_norm_weights:
                        nc.vector.tensor_tensor(fb[:], fb[:], whn_sb[:],
                                                op=OP.mult)
                    if which[0] == 'q':
                        nc.vector.tensor_scalar(qs_f32[:], fps[:], rstd[:],
                                                None, OP.mult)
                        if apply_norm_weights:
                            nc.vector.tensor_tensor(qs_f32[:], qs_f32[:],
                                                    whn_sb[:], op=OP.mult)
                        qhb = scr_bf.tile([128, D], BF16, tag="tmb")
                        nc.vector.tensor_add(qhb[:], fb[:], peq_sb[:])

                        def do_q_proj(qhb=qhb):
                            qht = scr_bf.tile([128, D], BF16, tag="tmb")
                            qht_v = qht[:].rearrange("p (g x) -> p g x", g=2)
                            for g in range(2):
                                transpose_to(
                                    qht_v[:, g:g + 1, :]
                                    .rearrange("p g x -> p (g x)")
                                    .rearrange("p (u x) -> p u x", u=4),
                                    qhb[:],
                                    [g * 4 + u for u in range(4)],
                                )
                            qps = mmps.tile([128, D], F32, tag="mm")
                            for j in range(NJ):
                                for h in range(2):
                                    nc.tensor.matmul(
                                        qps[:, h * 512:(h + 1) * 512],
                                        qht[:, j * 128:(j + 1) * 128],
                                        wq_sb[:, j * D + h * 512:
                                              j * D + (h + 1) * 512],
                                        start=(j == 0),
                                        stop=(j == NJ - 1),
                                    )
                            qrstd = rms_stats(qps[:])
                            qb = scr_bf.tile([128, D], BF16, tag="tmb")
                            nc.vector.tensor_scalar(qb[:], qps[:], qrstd[:],
                                                    None, OP.mult)
                            if apply_norm_weights:
                                nc.vector.tensor_tensor(qb[:], qb[:], wqn_sb[:],
                                                        op=OP.mult)
                            nc.sync.dma_start(ag_in[:], qb[:])
                            nc.gpsimd.collective_compute(
                                "AllGather", OP.bypass,
                                replica_groups=[list(range(NC))],
                                ins=[ag_in.opt()],
                                outs=[ag_out.opt()],
                            )
                        pending.append(do_q_proj)
                    else:
                        b, tl = which
                        # gpsimd DMA queue: keeps the scalar FIFO free so the
                        # next tile's rms Square isn't gated by this tile's fb
                        nc.gpsimd.dma_start(
                            fcache[(b * 4 + tl) * 128:(b * 4 + tl + 1) * 128, :],
                            fb[:])

                        def do_f_trans(b=b, tl=tl, fb=fb):
                            for g in range(2):
                                transpose_to(
                                    fusedT_bv[b][:, g * 4:(g + 1) * 4,
                                                 tl * 128:(tl + 1) * 128],
                                    fb[:],
                                    [g * 4 + u for u in range(4)],
                                )
                        pending.append(do_f_trans)

                fc_tile(('q',))
                flush_pending()   # q projection + AllGather fire immediately
                for b in range(B):
                    for tl in range(4):
                        fc_tile((b, tl))
                flush_pending()

            # =============== phase 2: K/V, attention, RS, epilogue ===========
            with tc.tile_pool(name="ph2", bufs=1) as ph2:
                kT_b = [ph2.tile([128, NJ * CH], BF16, name=f"kT{b}")
                        for b in range(B)]
                kT_bv = [kT[:].rearrange("p (j t) -> p j t", j=NJ)
                         for kT in kT_b]
                # token-major normalized fused, read back from DRAM: the
                # attnV rhs (V projection is folded into the epilogue since
                # (P @ fused) @ Wv == P @ (fused @ Wv))
                ftok_b = [ph2.tile([128, 4 * D], BF16, name=f"ftok{b}")
                          for b in range(B)]
                for b in range(B):
                    for tl in range(4):
                        nc.scalar.dma_start(
                            ftok_b[b][:, tl * D:(tl + 1) * D],
                            fcache[(b * 4 + tl) * 128:(b * 4 + tl + 1) * 128, :])
                qT = ph2.tile([128, NJ * QT], BF16)
                qT_v = qT[:].rearrange("p (j t) -> p j t", j=NJ)
                wv_sb = ph2.tile([128, NJ * D], BF16)
                nc.scalar.dma_start(wv_sb[:], wv_d.ap())
                wo_sb = ph2.tile([128, NJ * D], BF16)
                nc.scalar.dma_start(wo_sb[:], wo_d.ap())

                # precompute all 8 causal masks (depends only on iota/thr);
                # each is added onto the scores INSIDE the PSUM accumulation
                # via an identity matmul, so softmax needs no vector hop
                mb_all = ph2.tile([128, NC * CH], BF16)
                for ti in range(NC):
                    nc.vector.tensor_scalar(mb_all[:, ti * CH:(ti + 1) * CH],
                                            iota_sb[:], thr_sb[:, ti:ti + 1],
                                            MASK_NEG, OP.is_gt, OP.mult)

                pend2 = []

                def flush2():
                    while pend2:
                        pend2.pop(0)()

                def flush_keep(n):
                    while len(pend2) > n:
                        pend2.pop(0)()

                def k_tile(bb, tl):
                    khb = scr_bf.tile([128, NJ * 128], BF16, tag="khb")
                    nc.vector.tensor_add(
                        khb[:].rearrange("p (j x) -> p j x", j=NJ),
                        fusedT_bv[bb][:, :, tl * 128:(tl + 1) * 128],
                        pet_sb[:].rearrange("p (j t) -> p j t", j=NJ)
                        [:, :, tl * 128:(tl + 1) * 128],
                    )
                    kps = mmps.tile([128, D], F32, tag="mm")
                    for j in range(NJ):
                        for h in range(2):
                            nc.tensor.matmul(
                                kps[:, h * 512:(h + 1) * 512],
                                khb[:, j * 128:(j + 1) * 128],
                                wk_sb[:, j * D + h * 512: j * D + (h + 1) * 512],
                                start=(j == 0),
                                stop=(j == NJ - 1),
                            )
                    flush2()
                    krstd = rms_stats(kps[:])
                    kb = scr_bf.tile([128, D], BF16, tag="tmb")
                    nc.vector.tensor_scalar(kb[:], kps[:], krstd[:], None, OP.mult)
                    if apply_norm_weights:
                        nc.vector.tensor_tensor(kb[:], kb[:], wkn_sb[:],
                                                op=OP.mult)

                    def do_k_trans(bb=bb, tl=tl, kb=kb):
                        for g in range(2):
                            transpose_to(
                                kT_bv[bb][:, g * 4:(g + 1) * 4,
                                          tl * 128:(tl + 1) * 128],
                                kb[:],
                                [g * 4 + u for u in range(4)],
                            )
                    pend2.append(do_k_trans)

                # ag_out rows viewed as (core, 16-query block, slot)
                ag_v = ag_out[:].rearrange("(c k s) d -> c k s d", c=NC, k=8)

                def qt_tile(ti):
                    """Gather attn q-tile ti (16 queries from each core) in
                    one strided DMA (sync-engine descriptor time is scarce)."""
                    b, i = divmod(ti, 4)
                    qg = scr_bf.tile([128, D], BF16, tag="qg", bufs=2)
                    nc.sync.dma_start(qg[:], ag_v[:, b * 4 + i, :, :])

                    def do_qt_trans(ti=ti, qg=qg):
                        for g in range(2):
                            transpose_to(
                                qT_v[:, g * 4:(g + 1) * 4,
                                     ti * 128:(ti + 1) * 128],
                                qg[:],
                                [g * 4 + u for u in range(4)],
                            )
                    pend2.append(do_qt_trans)

                # attn tile -> (rs buffer, per-core row stride, row offset)
                RS_MAP = {(0, 0): (0, 32, 0), (0, 1): (0, 32, 16),
                          (0, 2): (1, 32, 0), (0, 3): (1, 32, 16),
                          (1, 0): (2, 48, 0), (1, 1): (2, 48, 16),
                          (1, 2): (2, 48, 32), (1, 3): (3, 16, 0)}

                def attn_tile(bb, i):
                    ti = bb * 4 + i
                    sps = scps.tile([128, 512], F32, tag="sc")
                    for j in range(NJ):
                        nc.tensor.matmul(
                            sps[:],
                            qT[:, j * QT + ti * 128: j * QT + (ti + 1) * 128],
                            kT_b[bb][:, j * CH:(j + 1) * CH],
                            start=(j == 0),
                            stop=False,
                        )
                    nc.tensor.matmul(          # sps += I @ mask
                        sps[:],
                        id_sb[:],
                        mb_all[:, ti * CH:(ti + 1) * CH],
                        start=False,
                        stop=True,
                    )
                    flush_keep(2)
                    o_sb = scr_f.tile([128, D + 1], F16, tag="osb")
                    lacc = stat.tile([128, 1], F32, tag="lacc")
                    probs = scr_bf.tile([128, CH], BF16, tag="probs")
                    nc.scalar.activation(probs[:], sps[:], AF.Exp, scale=SCALE,
                                         bias=ebias_sb[:], accum_out=lacc[:])
                    nc.vector.tensor_copy(o_sb[:, D:D + 1], lacc[:])

                    def do_attn_out(bb=bb, i=i, probs=probs, o_sb=o_sb):
                        pps = trps.tile([128, 512], BF16, tag="trp")
                        for u in range(4):
                            nc.tensor.transpose(
                                pps[:, u * 128:(u + 1) * 128],
                                probs[:, u * 128:(u + 1) * 128],
                                id_sb[:],
                            )
                        pT = scr_bf.tile([128, 512], BF16, tag="pT")
                        nc.vector.tensor_copy(pT[:], pps[:])
                        ops_ = mmps.tile([128, D], F32, tag="mm")
                        for u in range(4):
                            for h in range(2):
                                nc.tensor.matmul(
                                    ops_[:, h * 512:(h + 1) * 512],
                                    pT[:, u * 128:(u + 1) * 128],
                                    ftok_b[bb][:, u * D + h * 512:
                                               u * D + h * 512 + 512],
                                    start=(u == 0),
                                    stop=(u == 3),
                                )
                        nc.vector.tensor_copy(o_sb[:, 0:D], ops_[:])
                        ri, stride, off = RS_MAP[(bb, i)]
                        dst = rs_in[ri][:].rearrange(
                            "(c o s) d -> c o s d", c=NC, o=stride // 16)
                        nc.sync.dma_start(dst[:, off // 16, :, :], o_sb[:])
                    pend2.append(do_attn_out)

                def reduce_scatter(h):
                    nc.gpsimd.collective_compute(
                        "ReduceScatter", OP.add,
                        replica_groups=[list(range(NC))],
                        ins=[rs_in[h].opt()],
                        outs=[rs_out[h].opt()],
                    )

                def epilogue_half(bb):
                    # everything lives on partitions [bb*64, bb*64+64) so all
                    # elementwise ops have matching start partitions
                    p0 = bb * 64
                    sl = slice(p0, p0 + 64)
                    fo = scr_f.tile([128, D + 1], F16, tag="fo")
                    r0 = RS_ROWS[bb * 2] // NC
                    r1 = RS_ROWS[bb * 2 + 1] // NC
                    nc.sync.dma_start(fo[p0:p0 + r0, :], rs_out[bb * 2])
                    nc.sync.dma_start(fo[p0 + r0:p0 + r0 + r1, :],
                                      rs_out[bb * 2 + 1])
                    linv = stat.tile([128, 1], F32, tag="linv")
                    nc.vector.reciprocal(linv[sl, :], fo[sl, D:D + 1])

                    def trans64(dst, src, idm=id_sb, ps_dt=BF16):
                        for g in range(2):
                            ps = trps.tile([128, 512], ps_dt, tag="trp")
                            for u in range(4):
                                j = g * 4 + u
                                nc.tensor.transpose(
                                    ps[:, u * 64:(u + 1) * 64],
                                    src[sl, j * 128:(j + 1) * 128],
                                    idm[sl, p0:p0 + 64],
                                )
                            nc.vector.tensor_copy(dst[:, g * 256:(g + 1) * 256],
                                                  ps[:, 0:256])

                    aoT = scr_bf.tile([128, 512], BF16, tag="pT")
                    trans64(aoT, fo, idm=id16_sb, ps_dt=F16)
                    # V projection folded out of the per-chunk attention:
                    # apply Wv then Wo to this core's 64 rows only
                    z1 = mmps.tile([128, D], F32, tag="mm")
                    for j in range(NJ):
                        for h in range(2):
                            nc.tensor.matmul(
                                z1[sl, h * 512:(h + 1) * 512],
                                aoT[:, j * 64:(j + 1) * 64],
                                wv_sb[:, j * D + h * 512: j * D + (h + 1) * 512],
                                start=(j == 0),
                                stop=(j == NJ - 1),
                            )
                    z1b = scr_bf.tile([128, D], BF16, tag="tmb")
                    nc.scalar.copy(z1b[sl, :], z1[sl, :])
                    z1T = scr_bf.tile([128, 512], BF16, tag="pT")
                    trans64(z1T, z1b)
                    zps = mmps.tile([128, D], F32, tag="mm")
                    for j in range(NJ):
                        for h in range(2):
                            nc.tensor.matmul(
                                zps[sl, h * 512:(h + 1) * 512],
                                z1T[:, j * 64:(j + 1) * 64],
                                wo_sb[:, j * D + h * 512: j * D + (h + 1) * 512],
                                start=(j == 0),
                                stop=(j == NJ - 1),
                            )
                    # divide by l after Wo (row scaling commutes with @Wo)
                    hh = scr_f.tile([128, D], F32, tag="sqscr")
                    nc.vector.tensor_scalar(hh[sl, :], zps[sl, :], linv[sl, :],
                                            None, OP.mult)
                    nc.vector.tensor_tensor(hh[sl, :], hh[sl, :],
                                            qs_f32[sl, :], op=OP.add)
                    sq = scr_f.tile([128, D], F32, tag="sqscr")
                    ssq = stat.tile([128, 1], F32, tag="ssq2")
                    nc.scalar.activation(sq[sl, :], hh[sl, :], AF.Square,
                                         accum_out=ssq[sl, :])
                    ostd = stat.tile([128, 1], F32, tag="std2")
                    nc.scalar.activation(ostd[sl, :], ssq[sl, :], AF.Sqrt,
                                         scale=1.0 / D, bias=eps_sb[sl, :])
                    orstd = stat.tile([128, 1], F32, tag="rstd2")
                    nc.vector.reciprocal(orstd[sl, :], ostd[sl, :])
                    yv = scr_f.tile([128, D], F32, tag="yv", bufs=1)
                    nc.vector.tensor_scalar(yv[sl, :], hh[sl, :], orstd[sl, :],
                                            None, OP.mult)
                    if apply_norm_weights:
                        nc.vector.tensor_tensor(yv[sl, :], yv[sl, :],
                                                won_sb[sl, :], op=OP.mult)
                    nc.sync.dma_start(out.ap()[p0:p0 + 64, :], yv[sl, :])

                # ---- schedule ----
                # all K tiles first (scores only need kT), then V, so the
                # attention block (and its RS triggers) starts ~15us earlier
                for tl in range(4):
                    k_tile(0, tl)
                    qt_tile(2 * tl)
                    qt_tile(2 * tl + 1)
                flush2()        # kT(0,3) + qT tiles 6,7 before attn scores
                attn_tile(0, 0)
                attn_tile(0, 1)
                flush2()
                reduce_scatter(0)
                attn_tile(0, 2)
                attn_tile(0, 3)
                flush2()
                reduce_scatter(1)
                for tl in range(4):
                    k_tile(1, tl)
                flush2()        # kT(1,3) before batch-1 attn scores
                attn_tile(1, 0)
                attn_tile(1, 1)
                attn_tile(1, 2)
                flush2()
                reduce_scatter(2)
                attn_tile(1, 3)
                flush2()
                reduce_scatter(3)
                epilogue_half(0)   # PE work overlaps RS(3) wait
                epilogue_half(1)

    nc.compile()
    return nc


def _pe_table():
    half = D // 2
    inv_freq = np.exp(np.arange(half, dtype=np.float32)
                      * (-math.log(10000.0) / half))
    ang = np.arange(S, dtype=np.float32)[:, None] * inv_freq
    return np.concatenate([np.sin(ang), np.cos(ang)], axis=-1).astype(np.float32)


def _core_gidx(c):
    """Global query indices owned by core c (64 batch-0 then 64 batch-1)."""
    h = QPC // 2
    return np.concatenate([np.arange(c * h, (c + 1) * h),
                           N + np.arange(c * h, (c + 1) * h)])


def _pack_rows(a, nrow=128):
    """[R, C] with R = k*nrow -> [nrow, k*C] (block k at cols k*C)."""
    k = a.shape[0] // nrow
    return np.ascontiguousarray(
        a.reshape(k, nrow, a.shape[1]).transpose(1, 0, 2).reshape(nrow, -1))


def make_in_maps(np_inputs, apply_w=False):
    bf16 = ml_dtypes.bfloat16
    hid = np.asarray(np_inputs["hidden_states"], np.float32)
    pos = np.asarray(np_inputs["context_positions"])
    Wfc = np.asarray(np_inputs["W_fc"], np.float32)
    Wq = np.asarray(np_inputs["Wq"], np.float32)
    Wk = np.asarray(np_inputs["Wk"], np.float32)
    Wv = np.asarray(np_inputs["Wv"], np.float32)
    Wo = np.asarray(np_inputs["Wo"], np.float32)

    x = hid.reshape(B, S, LD)
    p = np.clip(pos.astype(np.int64), 0, S - 1)
    p_flat = p.reshape(QT)
    PE = _pe_table()

    wfc_p = _pack_rows(Wfc).astype(bf16)          # [128, 24*1024]
    wq_p = _pack_rows(Wq).astype(bf16)
    wk_p = _pack_rows(Wk).astype(bf16)
    wv_p = _pack_rows(Wv).astype(bf16)
    wo_p = _pack_rows(Wo).astype(bf16)

    iota_np = np.tile(np.arange(CH, dtype=np.float16), (128, 1))
    ident_np = np.eye(128, dtype=np.float32).astype(bf16)

    # thr[r, b*4+i] = position of query slot r in attn tile (b, i), minus
    # the core's chunk start.  Slot r = core (r//16)'s query (i*16 + r%16).
    rr = np.arange(128)

    in_maps = []
    for c in range(NC):
        sl = slice(c * CH, (c + 1) * CH)
        # xt col = (b*4+tl)*3072 + kk*128 + s  (tile-contiguous)
        xt_a = np.concatenate(
            [x[b, sl].reshape(4, 128, NKK, 128).transpose(3, 0, 2, 1)
             .reshape(128, NKK * CH) for b in range(B)],
            axis=1).astype(bf16)
        gidx = _core_gidx(c)
        xq_rows = x[gidx // N, p_flat[gidx]]              # [128, 3072]
        xqt_a = _pack_rows(np.ascontiguousarray(xq_rows.T)).astype(bf16)
        peq_a = PE[p_flat[gidx]].astype(bf16)
        pet_a = _pack_rows(np.ascontiguousarray(PE[sl].T)).astype(bf16)
        thr_a = np.empty((128, NC), np.float32)
        for b in range(B):
            for i in range(4):
                qidx = b * N + (rr // 16) * 64 + i * 16 + (rr % 16)
                thr_a[:, b * 4 + i] = p_flat[qidx].astype(np.float32) - c * CH
        m = {
            "xt": xt_a, "xqt": xqt_a,
            "wfc": wfc_p, "wq": wq_p, "wk": wk_p, "wv": wv_p, "wo": wo_p,
            "pet": pet_a, "peq": peq_a, "thr": thr_a,
            "iota": iota_np, "ident": ident_np,
        }
        if apply_w:
            m["whn"] = np.tile(np.asarray(np_inputs["w_hidden_norm"], np.float32), (128, 1))
            m["wqn"] = np.tile(np.asarray(np_inputs["w_q_norm"], np.float32), (128, 1))
            m["wkn"] = np.tile(np.asarray(np_inputs["w_k_norm"], np.float32), (128, 1))
            m["won"] = np.tile(np.asarray(np_inputs["w_out_norm"], np.float32), (128, 1))
        in_maps.append(m)
    return in_maps


def assemble_out(results):
    y = np.zeros((QT, D), np.float32)
    for c in range(NC):
        y[_core_gidx(c)] = results[c]["out"]
    return y.reshape(B, N, D)


def kernel(**inputs) -> np.ndarray:
    w_h = np.asarray(inputs["w_hidden_norm"], np.float32)
    w_q = np.asarray(inputs["w_q_norm"], np.float32)
    w_k = np.asarray(inputs["w_k_norm"], np.float32)
    w_o = np.asarray(inputs["w_out_norm"], np.float32)
    apply_w = not (np.all(w_h == 1) and np.all(w_q == 1)
                   and np.all(w_k == 1) and np.all(w_o == 1))

    key = ("nc", apply_w)
    if key not in _CACHE:
        _CACHE[key] = _build(apply_w)
    nc = _CACHE[key]

    in_maps = make_in_maps(inputs, apply_w)

    trace = os.environ.get("KERNEL_TRACE", "0") == "1"
    if trace:
        try:
            import axon_prof
            axon_prof.install()
        except Exception:
            trace = False
    res = run_bass_kernel_spmd(nc, in_maps, list(range(NC)), trace=trace)
    global LAST_EXEC_NS
    LAST_EXEC_NS = res.exec_time_ns

    return assemble_out(res.results).astype(np.float32)


LAST_EXEC_NS = None
